# revision 12
# baseline (speedup 1.0000x reference)
"""Trainium2 Bass kernel for the 5-layer LSTM + FC head (nn_LstmMemoryPredict).

Data-parallel over 8 NeuronCores: batch 256 -> 32 per core; the 5 LSTM layers
run as a wavefront (layer l processes time t = step - l), so each step does
5 layers x 32 batch = 160 independent lanes.

Per-step scheme (all gates through ONE sigmoid):
  gate order [i, f, o, g]; the g pre-activation is doubled host-side and the
  stored states are h~ = h/2, C = 2c, so that
    s   = sigmoid(z')            [128, 160]   (s_g = (tanh(z_g)+1)/2)
    d   = (s_g - 0.5) * s_i      = i*g/2      (DVE STT)
    e   = s_f * C                = 2*f*c      (GpSimd TT)
    C'  = 4*d + e                = 2*c_new    (DVE STT)
    tc  = sigmoid(C')            = (tanh(c)+1)/2
    h~  = (tc - 0.5) * s_o       = h/2        (DVE STT, bf16 -> h ring)
  All h~ consumers (Whh, Wih l>=1, W_fc) are pre-doubled host-side.
  Bias rides in a 33rd weight row against a constant-1.0 row of the h ring.
"""
import sys
sys.path.insert(0, "/opt/trn_rl_repo")

import numpy as np
import concourse.bass as bass
import concourse.bacc as bacc
import concourse.mybir as mybir
from concourse.tile import TileContext
from concourse.mybir import AluOpType, ActivationFunctionType

F32 = mybir.dt.float32
FP16 = mybir.dt.float16

H, L, D, BC = 32, 5, 64, 32     # hidden, layers, input dim, batch/core
F = L * BC                      # 160 free lanes per step
PRO = L - 1                     # 4 wavefront warm-up steps
GATE_PERM = np.r_[0:64, 96:128, 64:96]  # i,f,g,o -> i,f,o,g


def build(T: int, CHUNK: int):
    assert T % CHUNK == 0
    NB = T // CHUNK
    nc = bacc.Bacc(None, target_bir_lowering=False, debug=False)

    xt_d = nc.dram_tensor("xt", [D, (T + PRO) * BC], FP16, kind="ExternalInput")
    wh_d = nc.dram_tensor("wh", [H, L * 128], FP16, kind="ExternalInput")
    wx_d = nc.dram_tensor("wx", [H, (L - 1) * 128], FP16, kind="ExternalInput")
    w0_d = nc.dram_tensor("w0", [D, 128], FP16, kind="ExternalInput")
    b5_d = nc.dram_tensor("b5", [2 * L, 128], FP16, kind="ExternalInput")
    oneh_d = nc.dram_tensor("oneh", [2 * L, 2 * F], FP16, kind="ExternalInput")
    fcw_d = nc.dram_tensor("fcw", [H + 1, 1], FP16, kind="ExternalInput")
    out_d = nc.dram_tensor("out", [1, T * BC], F32, kind="ExternalOutput")

    with TileContext(nc) as tc:
        with (
            tc.tile_pool(name="wpool", bufs=1) as wpool,
            tc.tile_pool(name="state", bufs=1) as state,
            tc.tile_pool(name="xpool", bufs=2) as xpool,
            tc.tile_pool(name="spool", bufs=2, space="PSUM") as spool,
            tc.tile_pool(name="vpool", bufs=2) as vpool,
            tc.tile_pool(name="opool", bufs=2) as opool,
            tc.tile_pool(name="zpool", bufs=2, space="PSUM") as zpool,
            tc.tile_pool(name="fcpool", bufs=2, space="PSUM") as fcpool,
        ):
            # ---- weights ----
            wh = wpool.tile([H, L * 128], FP16)        # [32, 640]
            wx = wpool.tile([H, (L - 1) * 128], FP16)  # [32, 512]
            w0 = wpool.tile([D, 128], FP16)
            b5 = wpool.tile([2 * L, 128], FP16)        # hi/lo bias split
            oneh = wpool.tile([2 * L, 2 * F], FP16)
            fcw = wpool.tile([H + 1, 1], FP16)
            xpro = wpool.tile([D, PRO, BC], FP16)
            nc.sync.dma_start(wh[:], wh_d[:, :])
            nc.sync.dma_start(wx[:], wx_d[:, :])
            nc.sync.dma_start(w0[:], w0_d[:, :])
            nc.sync.dma_start(b5[:], b5_d[:, :])
            nc.sync.dma_start(oneh[:], oneh_d[:, :])
            nc.sync.dma_start(fcw[:], fcw_d[:, :])
            nc.sync.dma_start(xpro[:], xt_d[:, 0:PRO * BC])

            # ---- persistent state ----
            # h ring: rows 0:32 h~ (bf16), row 32 == 1.0 (bias row)
            hs = state.tile([H + 1, CHUNK, F], FP16)
            C = state.tile([H, F], F32)                # C = 2c
            nc.gpsimd.memset(hs[:], 0.0)
            nc.gpsimd.memset(hs[H:H + 1, :, :], 1.0)
            nc.gpsimd.memset(C[:], 0.0)

            def whl(l):
                return wh[:, l * 128:(l + 1) * 128]

            def wxl(l):  # l in 1..4
                return wx[:, (l - 1) * 128:(l + 1 - 1) * 128]

            def emit_step(zb, k2, slot_w, slot_r, x_mv, mask_from=None):
                z = zb[:, k2 * F:(k2 + 1) * F]
                if k2 == 0:  # bias for both steps of this bank; sole start
                    nc.tensor.matmul(zb[:, :], b5[:], oneh[:], start=True,
                                     stop=False, skip_group_check=True)
                # layer-0 x projection: no h dependency, can run early
                nc.tensor.matmul(z[:, 0:BC], w0[:], x_mv, start=False,
                                 stop=False, skip_group_check=True)
                for l in range(L):  # recurrent projections
                    nc.tensor.matmul(z[:, l * BC:(l + 1) * BC], whl(l),
                                     hs[0:H, slot_r, l * BC:(l + 1) * BC],
                                     start=False, stop=False,
                                     skip_group_check=True)
                for l in range(1, L):  # inter-layer projections
                    nc.tensor.matmul(z[:, l * BC:(l + 1) * BC], wxl(l),
                                     hs[0:H, slot_r, (l - 1) * BC:l * BC],
                                     start=False, stop=(l == L - 1),
                                     skip_group_check=True)
                # activations: i,f,o sigmoid (stays in PSUM: mixed-base reads
                # from PSUM are legal), g tanh into a base-0 SBUF tile
                s = spool.tile([3 * H, F], F32, tag="s")
                nc.scalar.activation(s[:], z[0:3 * H, :],
                                     ActivationFunctionType.Sigmoid)
                g = vpool.tile([H, F], F32, tag="g")
                nc.scalar.activation(g[:], z[3 * H:4 * H, :],
                                     ActivationFunctionType.Tanh)
                # cell update: C = 2*(i*g) + f*C   (C == 2c)
                m = vpool.tile([H, F], F32, tag="m")
                e = vpool.tile([H, F], F32, tag="e")
                nc.vector.tensor_tensor(e[:], C[:], s[H:2 * H, :], AluOpType.mult)
                nc.vector.tensor_tensor(m[:], g[:], s[0:H, :], AluOpType.mult)
                nc.vector.scalar_tensor_tensor(
                    C[:], m[:], 2.0, e[:], AluOpType.mult, AluOpType.add)
                tcn = vpool.tile([H, F], F32, tag="tc")
                nc.scalar.activation(tcn[:], C[:], ActivationFunctionType.Sigmoid)
                nc.vector.scalar_tensor_tensor(
                    hs[0:H, slot_w, :], tcn[:], 0.5, s[2 * H:3 * H, :],
                    AluOpType.subtract, AluOpType.mult)
                if mask_from is not None:
                    nc.gpsimd.memset(hs[0:H, slot_w, mask_from * BC:F], 0.0)
                    nc.gpsimd.memset(C[:, mask_from * BC:F], 0.0)

            # ---- prologue: wavefront warm-up, steps s=0..PRO-1 ----
            zb = None
            for s_ in range(PRO):
                if s_ % 2 == 0:
                    zb = zpool.tile([128, 2 * F], F32, tag="zb")
                emit_step(zb, s_ % 2, (CHUNK - PRO + s_) % CHUNK,
                          (CHUNK - PRO + s_ - 1) % CHUNK,
                          xpro[:, s_, :], mask_from=s_ + 1)

            # ---- main loop over chunks ----
            FCN = CHUNK * BC // 512  # FC matmuls per chunk (512 cols each)
            SPF = 512 // BC          # steps per FC matmul
            with tc.For_i(0, NB) as i:
                xb = xpool.tile([D, CHUNK, BC], FP16)
                nc.sync.dma_start(
                    xb[:], xt_d[:, bass.ds(i * (CHUNK * BC) + PRO * BC,
                                           CHUNK * BC)])
                ost = opool.tile([1, CHUNK * BC], F32)

                def fc_block(q):
                    fps = fcpool.tile([1, 512], F32, tag="fps")
                    nc.tensor.matmul(fps[:, :], fcw[:],
                                     hs[:, q * SPF:(q + 1) * SPF,
                                        (L - 1) * BC:F],
                                     start=True, stop=True,
                                     skip_group_check=True)
                    nc.scalar.copy(ost[:, q * 512:(q + 1) * 512], fps[:, :])

                for sl in range(CHUNK):
                    if sl % 2 == 0:
                        zb = zpool.tile([128, 2 * F], F32, tag="zb")
                    emit_step(zb, sl % 2, sl, (sl - 1) % CHUNK, xb[:, sl, :])
                    if (sl + 1) % SPF == 0:
                        fc_block((sl + 1) // SPF - 1)
                nc.sync.dma_start(out_d[:, bass.ds(i * (CHUNK * BC),
                                                   CHUNK * BC)], ost[:, :])

    nc.compile()
    return nc


# ---------------- host-side packing ----------------

def prep_weights(W_ih0, W_ih_rest, W_hh, b_ih, b_hh, W_fc, b_fc):
    mm_np = np.float16
    p = GATE_PERM
    wh_blocks, wx_blocks = [], []
    bias = np.zeros((L, 128), np.float32)
    for l in range(L):
        bias[l] = (b_ih[l] + b_hh[l])[p]
        wh_blocks.append(2.0 * W_hh[l][p].T)         # [32, 128]
        if l >= 1:
            wx_blocks.append(2.0 * W_ih_rest[l - 1][p].T)
    wh = np.concatenate(wh_blocks, axis=1).astype(mm_np)   # [32, 640]
    wx = np.concatenate(wx_blocks, axis=1).astype(mm_np)   # [32, 512]
    w0 = W_ih0[p].T.astype(mm_np)                          # [64, 128]
    # exact bias via hi+lo bf16 split: rows 2l (hi) and 2l+1 (lo)
    b_hi = bias.astype(mm_np).astype(np.float32)
    b_lo = bias - b_hi
    b5 = np.zeros((2 * L, 128), np.float32)
    b5[0::2] = b_hi
    b5[1::2] = b_lo
    oneh = np.zeros((2 * L, 2 * F), np.float32)
    for l in range(L):
        for k2 in range(2):
            oneh[2 * l:2 * l + 2,
                 k2 * F + l * BC:k2 * F + (l + 1) * BC] = 1.0
    fcw = np.concatenate([2.0 * W_fc.reshape(H, 1),
                          b_fc.reshape(1, 1)], axis=0).astype(mm_np)
    return {"wh": wh, "wx": wx, "w0": w0, "b5": b5.astype(mm_np),
            "oneh": oneh.astype(mm_np), "fcw": fcw}


def prep_x_core(x_core, T):
    # x_core [BC, T, D] fp32 -> xt [64, (T+PRO)*BC], col = t*BC + b, zero tail
    xt = np.zeros((D, (T + PRO) * BC), np.float32)
    xt[:, :T * BC] = x_core.transpose(2, 1, 0).reshape(D, T * BC)
    return xt.astype(np.float16)


# ---------------- public entry point ----------------
T_FULL, CHUNK_FULL, N_CORES = 2048, 32, 8
_NC_CACHE = {}


def _get_nc():
    if "nc" not in _NC_CACHE:
        _NC_CACHE["nc"] = build(T_FULL, CHUNK_FULL)
    return _NC_CACHE["nc"]


def kernel(x, W_ih0, W_ih_rest, W_hh, b_ih, b_hh, W_fc, b_fc):
    from concourse.bass_utils import run_bass_kernel_spmd
    nc = _get_nc()
    w = prep_weights(np.asarray(W_ih0), np.asarray(W_ih_rest),
                     np.asarray(W_hh), np.asarray(b_ih), np.asarray(b_hh),
                     np.asarray(W_fc), np.asarray(b_fc))
    x = np.asarray(x)
    in_maps = []
    for c in range(N_CORES):
        xs = x[c * BC:(c + 1) * BC]
        in_maps.append(dict(w, xt=prep_x_core(xs, T_FULL)))
    res = run_bass_kernel_spmd(nc, in_maps, core_ids=list(range(N_CORES)))
    outs = []
    for c in range(N_CORES):
        o = res.results[c]["out"].reshape(T_FULL, BC).T[:, :, None]
        outs.append(o)
    return np.concatenate(outs, axis=0).astype(np.float32)


# revision 19
# speedup vs baseline: 61.5309x; 61.5309x over previous
"""Trainium2 Bass kernel for the 5-layer LSTM + FC head (nn_LstmMemoryPredict).

Data-parallel over 8 NeuronCores: batch 256 -> 32 per core; the 5 LSTM layers
run as a wavefront (layer l processes time t = step - l), so each step does
5 layers x 32 batch = 160 independent lanes.

Measured HW costs (this platform) are dominated by per-instruction latency
(MM+LDW ~0.45us, ACT ~1.3us, DVE ~0.6us in dependency chains), so the step
is built for MINIMUM instruction count on the serial chain:

 - 6 matmuls/step: layer>=1 uses one K-stacked weight [2*Whh_l; bias_l;
   2*Wih_l] [65,128] against an h-ring slot holding [h~; 1.0; h~ shifted one
   layer-block]; layer 0 = wh_0 [33,128] (+bias row) and the bf16 x-proj w0.
 - ONE sigmoid for all 4 gates: gate order [i,f,o,g] with the g-gate
   pre-activations doubled host-side (tanh(x) = 2*sigmoid(2x)-1) and cell
   state stored as C = 2c so tanh(c) = 2*sigmoid(C)-1:
     s    = sigmoid(z')       [128,160] SBUF  (s_g = (g+1)/2)
     gc   = s_g - 0.5         (= g/2; base-0 realign, DVE tensor_scalar)
     m    = gc * s_i          (= i*g/2)
     e    = C * s_f           (= 2*f*c)
     C'   = 4*m + e           (= 2*c_new)
     tc   = sigmoid(C')       (= (tanh(c)+1)/2)
     h~   = (tc-0.5) * s_o    (= h/2; all h~ consumers pre-doubled)
   plus a second h~ write into the shifted rows of the ring.
 - h-path entirely fp32 (bf16 h storage alone costs ~2e-2 rel err); x path
   bf16 (negligible error).
 - vector-op base-partition rule (all SBUF operands of one op must share the
   start partition) is satisfied by placing: gates i@0/f@32/o@64/sg@96 in s,
   gc@0, m@0+e@32 in one tile, C@32, tc@64.
"""
import sys
sys.path.insert(0, "/opt/trn_rl_repo")

import numpy as np
import concourse.bass as bass
import concourse.bacc as bacc
import concourse.mybir as mybir
from concourse.tile import TileContext
from concourse.mybir import AluOpType, ActivationFunctionType

F32 = mybir.dt.float32
BF16 = mybir.dt.bfloat16

H, L, D, BC = 32, 5, 64, 32     # hidden, layers, input dim, batch/core
F = L * BC                      # 160 free lanes per step
PRO = L - 1                     # 4 wavefront warm-up steps
GATE_PERM = np.r_[0:64, 96:128, 64:96]  # i,f,g,o -> i,f,o,g


def build(T: int, CHUNK: int, reps: int = 1):
    assert T % CHUNK == 0
    NB = T // CHUNK
    nc = bacc.Bacc(None, target_bir_lowering=False, debug=False)

    xt_d = nc.dram_tensor("xt", [D, (T + PRO) * BC], BF16, kind="ExternalInput")
    wh0_d = nc.dram_tensor("wh0", [2 * H + 1, 128], F32, kind="ExternalInput")
    wstk_d = nc.dram_tensor("wstk", [2 * H + 1, (L - 1) * 128], F32,
                            kind="ExternalInput")
    w0_d = nc.dram_tensor("w0", [D, 128], BF16, kind="ExternalInput")
    fcw_d = nc.dram_tensor("fcw", [2 * H + 1, 1], F32, kind="ExternalInput")
    out_d = nc.dram_tensor("out", [1, T * BC], F32, kind="ExternalOutput")

    with TileContext(nc) as tc:
        with (
            tc.tile_pool(name="wpool", bufs=1) as wpool,
            tc.tile_pool(name="state", bufs=1) as state,
            tc.tile_pool(name="xpool", bufs=2) as xpool,
            tc.tile_pool(name="spool", bufs=2) as spool,
            tc.tile_pool(name="vpool", bufs=2) as vpool,
            tc.tile_pool(name="opool", bufs=2) as opool,
            tc.tile_pool(name="zpool", bufs=2, space="PSUM") as zpool,
            tc.tile_pool(name="fcpool", bufs=2, space="PSUM") as fcpool,
        ):
            # ---- weights ----
            wh0 = wpool.tile([2 * H + 1, 128], F32)
            wstk = wpool.tile([2 * H + 1, (L - 1) * 128], F32)
            w0 = wpool.tile([D, 128], BF16)
            fcw = wpool.tile([2 * H + 1, 1], F32)
            xpro = wpool.tile([D, PRO, BC], BF16)
            nc.sync.dma_start(wh0[:], wh0_d[:, :])
            nc.sync.dma_start(wstk[:], wstk_d[:, :])
            nc.sync.dma_start(w0[:], w0_d[:, :])
            nc.sync.dma_start(fcw[:], fcw_d[:, :])
            nc.sync.dma_start(xpro[:], xt_d[:, 0:PRO * BC])

            # ---- persistent state ----
            # h ring: rows 0:32 h~, rows 32:64 h~ shifted one layer-block
            # right, row 64 == 1.0 (so [0:65] is the K-stacked matmul rhs)
            hs = state.tile([2 * H + 1, CHUNK, F], F32)
            CT = state.tile([2 * H, F], F32)           # rows 32:64 = C (2c)
            nc.gpsimd.memset(hs[:], 0.0)
            nc.gpsimd.memset(hs[2 * H:2 * H + 1, :, :], 1.0)
            nc.gpsimd.memset(CT[:], 0.0)
            C = CT[H:2 * H, :]

            def wstkl(l):  # l in 1..4
                return wstk[:, (l - 1) * 128:l * 128]

            def emit_step(zb, k2, slot_w, slot_r, x_mv, mask_from=None,
                          bulk_x=False):
                z = zb[:, k2, :]
                if k2 == 0:
                    # x projection for BOTH steps of this bank; it only needs
                    # the x DMA, so it runs early and is the bank's sole
                    # start=True (pending-zero marks make later blocks
                    # overwrite-clean)
                    if bulk_x:
                        nc.tensor.matmul(zb[:, :, 0:BC], w0[:], x_mv,
                                         start=True, stop=False,
                                         skip_group_check=True)
                nc.tensor.matmul(z[:, 0:BC], wh0[:],
                                 hs[0:2 * H + 1, slot_r, 0:BC],
                                 start=(k2 == 0 and not bulk_x), stop=False,
                                 skip_group_check=True)
                if not bulk_x:
                    nc.tensor.matmul(z[:, 0:BC], w0[:], x_mv, start=False,
                                     stop=False, skip_group_check=True)
                for l in range(1, L):  # K-stacked [2Whh_l; 2Wih_l; b_l]
                    nc.tensor.matmul(z[:, l * BC:(l + 1) * BC], wstkl(l),
                                     hs[0:2 * H + 1, slot_r,
                                        l * BC:(l + 1) * BC],
                                     start=False, stop=(l == L - 1),
                                     skip_group_check=True)
                # ONE sigmoid for all four gate blocks -> SBUF
                s = spool.tile([128, F], F32, tag="s")
                nc.scalar.activation(s[:], z[:, :],
                                     ActivationFunctionType.Sigmoid)
                # cell update (bases: gc@0, m/e both @32 in me, C@32, tc@64)
                gc = vpool.tile([H, F], F32, tag="gc")
                me = vpool.tile([2 * H, 2 * F], F32, tag="me")
                tcn = vpool.tile([3 * H, F], F32, tag="tc")
                nc.vector.tensor_scalar(gc[:], s[3 * H:4 * H, :], 0.5, None,
                                        AluOpType.subtract)
                nc.vector.tensor_tensor(me[H:2 * H, F:2 * F], C,
                                        s[H:2 * H, :], AluOpType.mult)
                nc.vector.tensor_tensor(me[H:2 * H, 0:F], gc[:], s[0:H, :],
                                        AluOpType.mult)
                nc.vector.scalar_tensor_tensor(
                    C, me[H:2 * H, 0:F], 4.0, me[H:2 * H, F:2 * F],
                    AluOpType.mult, AluOpType.add)
                nc.scalar.activation(tcn[2 * H:3 * H, :], C,
                                     ActivationFunctionType.Sigmoid)
                nc.vector.scalar_tensor_tensor(
                    hs[0:H, slot_w, :], tcn[2 * H:3 * H, :], 0.5,
                    s[2 * H:3 * H, :], AluOpType.subtract, AluOpType.mult)
                nc.vector.scalar_tensor_tensor(
                    hs[H:2 * H, slot_w, BC:F],
                    tcn[2 * H:3 * H, 0:F - BC], 0.5,
                    s[2 * H:3 * H, 0:F - BC],
                    AluOpType.subtract, AluOpType.mult)
                if mask_from is not None:
                    nc.gpsimd.memset(hs[0:H, slot_w, mask_from * BC:F], 0.0)
                    if mask_from + 1 < L:
                        nc.gpsimd.memset(
                            hs[H:2 * H, slot_w,
                               (mask_from + 1) * BC:F], 0.0)
                    nc.gpsimd.memset(C[:, mask_from * BC:F], 0.0)

            # ---- prologue: wavefront warm-up, steps s=0..PRO-1 ----
            zb = None
            for s_ in range(PRO):
                if s_ % 2 == 0:
                    zb = zpool.tile([128, 2, F], F32, tag="zb")
                emit_step(zb, s_ % 2, (CHUNK - PRO + s_) % CHUNK,
                          (CHUNK - PRO + s_ - 1) % CHUNK,
                          xpro[:, s_, :], mask_from=s_ + 1)

            # ---- main loop over chunks (reps>1: timing-only rebuild) ----
            SPF = 512 // BC          # steps per FC matmul
            for _rep in range(reps):
              with tc.For_i(0, NB) as i:
                xb = xpool.tile([D, CHUNK, BC], BF16)
                nc.sync.dma_start(
                    xb[:], xt_d[:, bass.ds(i * (CHUNK * BC) + PRO * BC,
                                           CHUNK * BC)])
                ost = opool.tile([1, CHUNK * BC], F32)

                def fc_block(q):
                    fps = fcpool.tile([1, 512], F32, tag="fps")
                    nc.tensor.matmul(fps[:, :], fcw[:],
                                     hs[0:2 * H + 1, q * SPF:(q + 1) * SPF,
                                        (L - 1) * BC:F],
                                     start=True, stop=True,
                                     skip_group_check=True)
                    nc.vector.tensor_copy(ost[:, q * 512:(q + 1) * 512],
                                          fps[:, :])

                for sl in range(CHUNK):
                    if sl % 2 == 0:
                        zb = zpool.tile([128, 2, F], F32, tag="zb")
                    emit_step(zb, sl % 2, sl, (sl - 1) % CHUNK,
                              xb[:, sl:sl + 2, :] if sl % 2 == 0 else None,
                              bulk_x=True)
                    if (sl + 1) % SPF == 0:
                        fc_block((sl + 1) // SPF - 1)
                nc.sync.dma_start(out_d[:, bass.ds(i * (CHUNK * BC),
                                                   CHUNK * BC)], ost[:, :])

    nc.compile()
    return nc


# ---------------- host-side packing ----------------

def prep_weights(W_ih0, W_ih_rest, W_hh, b_ih, b_hh, W_fc, b_fc):
    import ml_dtypes
    p = GATE_PERM
    gsc = np.ones((1, 128), np.float32)
    gsc[0, 96:128] = 2.0            # g-gate pre-activation x2 (sigmoid trick)
    bias = [((b_ih[l] + b_hh[l])[p] * gsc[0])[None, :] for l in range(L)]
    zer = np.zeros((H, 128), np.float32)
    wh0 = np.vstack([2.0 * W_hh[0][p].T * gsc, zer, bias[0]]).astype(np.float32)
    stk = [np.vstack([2.0 * W_hh[l][p].T * gsc,
                      2.0 * W_ih_rest[l - 1][p].T * gsc, bias[l]])
           for l in range(1, L)]
    wstk = np.concatenate(stk, axis=1).astype(np.float32)   # [65, 512]
    w0 = (W_ih0[p].T * gsc).astype(ml_dtypes.bfloat16)      # [64, 128]
    fcw = np.concatenate([2.0 * W_fc.reshape(H, 1), np.zeros((H, 1), np.float32),
                          b_fc.reshape(1, 1)], axis=0).astype(np.float32)
    return {"wh0": wh0, "wstk": wstk, "w0": w0, "fcw": fcw}


def prep_x_core(x_core, T):
    import ml_dtypes
    # x_core [BC, T, D] fp32 -> xt [64, (T+PRO)*BC], col = t*BC + b, zero tail
    xt = np.zeros((D, (T + PRO) * BC), np.float32)
    xt[:, :T * BC] = x_core.transpose(2, 1, 0).reshape(D, T * BC)
    return xt.astype(ml_dtypes.bfloat16)


# ---------------- public entry point ----------------
T_FULL, CHUNK_FULL, N_CORES = 2048, 32, 8
_NC_CACHE = {}


def _get_nc():
    if "nc" not in _NC_CACHE:
        _NC_CACHE["nc"] = build(T_FULL, CHUNK_FULL)
    return _NC_CACHE["nc"]


def kernel(x, W_ih0, W_ih_rest, W_hh, b_ih, b_hh, W_fc, b_fc):
    from concurrent.futures import ThreadPoolExecutor
    from concourse.bass_utils import run_bass_kernel_spmd
    nc = _get_nc()
    w = prep_weights(np.asarray(W_ih0), np.asarray(W_ih_rest),
                     np.asarray(W_hh), np.asarray(b_ih), np.asarray(b_hh),
                     np.asarray(W_fc), np.asarray(b_fc))
    x = np.asarray(x)
    with ThreadPoolExecutor(max_workers=8) as ex:
        xts = list(ex.map(
            lambda c: prep_x_core(x[c * BC:(c + 1) * BC], T_FULL),
            range(N_CORES)))
    in_maps = [dict(w, xt=xts[c]) for c in range(N_CORES)]
    res = run_bass_kernel_spmd(nc, in_maps, core_ids=list(range(N_CORES)))
    outs = []
    for c in range(N_CORES):
        o = res.results[c]["out"].reshape(T_FULL, BC).T[:, :, None]
        outs.append(o)
    return np.concatenate(outs, axis=0).astype(np.float32)
